# Initial kernel scaffold
#
import sys, time
sys.path.insert(0, "/opt/trn_rl_repo")
import numpy as np
from concourse import bass, bacc, mybir, tile
from concourse.bass_utils import run_bass_kernel_spmd

# Problem constants (nn_Memory_88656714925588)
B, CK, CV = 1, 64, 256
H, W, T = 64, 64, 8
NE = H * W * T            # 32768
Q = H * W                 # 4096
NC = 8                    # cores
NE_LOC = NE // NC         # 4096 memory elements per core
Q_LOC = Q // NC           # 512 queries per core in phase 3
TOPK = 20
NGRP = 3                  # groups per query-tile in phase 1
GB = [0, 1366, 2732, 4096]  # uneven group bounds over NE_LOC
NCAND = NGRP * 8          # 24 candidates per (query, core)
NSLOT = NC * NCAND        # 256 candidates per query after all-gather
NQT = Q // 128            # 32 query tiles in phase 1
NQT3 = Q_LOC // 128       # 4 query tiles per core in phase 3
F32 = mybir.dt.float32
U32 = mybir.dt.uint32
NEG = -1e30

_prog_cache = {}


def _build_program(phases="123"):
    if phases in _prog_cache:
        return _prog_cache[phases]
    nc = bacc.Bacc()
    qTa = nc.dram_tensor("qTa", [CK + 1, Q], F32, kind="ExternalInput")
    mkA = nc.dram_tensor("mkA", [CK + 1, NE_LOC], F32, kind="ExternalInput")
    vT = nc.dram_tensor("vT", [NE, 2 * CV], F32, kind="ExternalInput")
    gnc = nc.dram_tensor("gnc", [128, NCAND], F32, kind="ExternalInput")
    prow256 = nc.dram_tensor("prow256", [128, 1], F32, kind="ExternalInput")
    out = nc.dram_tensor("out", [Q_LOC, 2 * CV], F32, kind="ExternalOutput")

    with tile.TileContext(nc) as tc:
        with tc.tile_pool(name="sbuf", bufs=2) as pool, \
             tc.tile_pool(name="deep", bufs=10) as deep, \
             tc.tile_pool(name="affp", bufs=4) as affp, \
             tc.tile_pool(name="cst", bufs=1) as cst, \
             tc.tile_pool(name="psum", bufs=2, space="PSUM") as psum, \
             tc.tile_pool(name="dram", bufs=2, space="DRAM") as dram:

            qt = cst.tile([CK + 1, Q], F32)
            mkt = cst.tile([CK + 1, NE_LOC], F32)
            # chunked loads: first matmul needs only mkt[:, :512] and
            # qt[:, :128], so let compute start before the full MB lands
            for ci in range(8):
                nc.sync.dma_start(
                    out=mkt[:, ci * 512:(ci + 1) * 512],
                    in_=mkA[:, ci * 512:(ci + 1) * 512])
            for ci in range(4):
                nc.sync.dma_start(
                    out=qt[:, ci * 1024:(ci + 1) * 1024],
                    in_=qTa[:, ci * 1024:(ci + 1) * 1024])
            gb = cst.tile([128, NCAND], F32)
            nc.sync.dma_start(out=gb[:], in_=gnc[:])
            pr256 = cst.tile([128, 1], F32)
            nc.sync.dma_start(out=pr256[:], in_=prow256[:])

            candL = dram.tile([Q, 2 * NCAND], F32)
            candX = dram.tile([Q, 2 * NCAND], F32)

            # ---------------- Phase 3: merge + readout (q-sharded) --------
            def phase3(tt):
                cG = pool.tile([128, NC * 2 * NCAND], F32, tag="cG")
                nc.sync.dma_start(
                    out=cG[:],
                    in_=candX[tt * NC * 128:(tt + 1) * NC * 128, :]
                    .rearrange("(g p) c -> p g c", p=128))
                candQ = dram.tile([128 * NSLOT, 2], F32, tag="candQ")
                nc.sync.dma_start(
                    out=candQ[:].rearrange("(p u) two -> p (u two)", p=128),
                    in_=cG[:])
                # exact merge: 3 rounds of top-8 on the strided value view
                sv = cG[:].rearrange("p (u two) -> p u two", two=2)[:, :, 0]
                gvals = pool.tile([128, 24], F32, tag="gvals")
                gpos = pool.tile([128, 24], U32, tag="gpos")
                for r in range(3):
                    m8 = gvals[:, r * 8:(r + 1) * 8]
                    nc.vector.max(out=m8, in_=sv)
                    nc.vector.max_index(
                        out=gpos[:, r * 8:(r + 1) * 8], in_max=m8, in_values=sv)
                    if r < 2:
                        nc.vector.match_replace(
                            out=sv, in_to_replace=m8, in_values=sv, imm_value=NEG)
                # softmax over the top-20 values
                negm = pool.tile([128, 1], F32, tag="negm")
                nc.vector.tensor_scalar(
                    negm[:], gvals[:, 0:1], -1.0, None, op0=mybir.AluOpType.mult)
                wexp = pool.tile([128, TOPK], F32, tag="wexp")
                ssum = pool.tile([128, 1], F32, tag="ssum")
                nc.scalar.activation(
                    out=wexp[:], in_=gvals[:, :TOPK],
                    func=mybir.ActivationFunctionType.Exp,
                    bias=negm[:], scale=1.0, accum_out=ssum[:])
                rs = pool.tile([128, 1], F32, tag="rs")
                nc.vector.reciprocal(rs[:], ssum[:])
                wgt = pool.tile([128, TOPK], F32, tag="wgt")
                nc.vector.tensor_scalar(
                    wgt[:], wexp[:], rs[:], None, op0=mybir.AluOpType.mult)
                # winner pair offsets: row p of candQ-pairs = p*256 + pos
                posf = pool.tile([128, 24], F32, tag="posf")
                nc.vector.tensor_copy(posf[:], gpos[:])
                nc.vector.tensor_scalar(
                    posf[:], posf[:], pr256[:], None, op0=mybir.AluOpType.add)
                pou = pool.tile([128, 24], U32, tag="pou")
                nc.vector.tensor_copy(pou[:], posf[:])
                acc = pool.tile([128, 2 * CV], F32, tag="acc")
                nc.vector.memset(acc[:], 0.0)
                for k in range(TOPK):
                    pk = deep.tile([128, 2], F32, tag="pk")
                    nc.gpsimd.indirect_dma_start(
                        out=pk[:], out_offset=None, in_=candQ[:],
                        in_offset=bass.IndirectOffsetOnAxis(
                            ap=pou[:, k:k + 1], axis=0))
                    iku = deep.tile([128, 1], U32, tag="iku")
                    nc.scalar.copy(out=iku[:], in_=pk[:, 1:2])
                    gk = deep.tile([128, 2 * CV], F32, tag="gk")
                    nc.gpsimd.indirect_dma_start(
                        out=gk[:], out_offset=None, in_=vT[:],
                        in_offset=bass.IndirectOffsetOnAxis(ap=iku[:], axis=0))
                    nc.vector.scalar_tensor_tensor(
                        out=acc[:], in0=gk[:], scalar=wgt[:, k:k + 1], in1=acc[:],
                        op0=mybir.AluOpType.mult, op1=mybir.AluOpType.add)
                nc.sync.dma_start(
                    out=out[tt * 128:(tt + 1) * 128, :], in_=acc[:])


            # ---------------- Phase 1: local affinity + per-group top-8 ----
            # tile order: chunk-major (j, d) with t = d*NQT3 + j so each
            # chunk's AllToAll can fire as soon as its 8 tiles are done
            _order = [d * NQT3 + j for j in range(NQT3) for d in range(NC)]
            for ti, t in enumerate(_order[:NQT if "1" in phases else 0]):
                affs = affp.tile([128, NE_LOC], F32, tag="affs")
                cvals = pool.tile([128, NCAND], F32, tag="cvals", bufs=4)
                cidx = pool.tile([128, NCAND], U32, tag="cidx", bufs=4)
                for h in range(2):
                    ph = psum.tile([128, NE_LOC // 2], F32, tag="ph")
                    for c in range(4):
                        nc.tensor.matmul(
                            out=ph[:, c * 512:(c + 1) * 512],
                            lhsT=qt[:, t * 128:(t + 1) * 128],
                            rhs=mkt[:, h * 2048 + c * 512: h * 2048 + (c + 1) * 512],
                            start=True, stop=True)
                    nc.scalar.copy(out=affs[:, h * 2048:(h + 1) * 2048], in_=ph[:])
                    for g in range(NGRP):
                        if not (GB[g] < (h + 1) * 2048 and GB[g + 1] > h * 2048
                                and GB[g + 1] <= (h + 1) * 2048):
                            continue
                        sl = affs[:, GB[g]:GB[g + 1]]
                        nc.vector.max(out=cvals[:, g * 8:(g + 1) * 8], in_=sl)
                        nc.vector.max_index(
                            out=cidx[:, g * 8:(g + 1) * 8],
                            in_max=cvals[:, g * 8:(g + 1) * 8], in_values=sl)
                crow = pool.tile([128, 2 * NCAND], F32, tag="crow", bufs=4)
                cr3 = crow[:].rearrange("p (u two) -> p u two", two=2)
                nc.scalar.copy(out=cr3[:, :, 0], in_=cvals[:])
                nc.vector.scalar_tensor_tensor(
                    out=cr3[:, :, 1], in0=cidx[:], scalar=1.0, in1=gb[:],
                    op0=mybir.AluOpType.mult, op1=mybir.AluOpType.add)
                j, d = t % NQT3, t // NQT3
                row = (j * NC + d) * 128
                nc.sync.dma_start(
                    out=candL[row:row + 128, :], in_=crow[:])
                if "2" in phases and ti % NC == NC - 1:
                    nc.gpsimd.collective_compute(
                        "AllToAll", mybir.AluOpType.bypass,
                        replica_groups=[list(range(NC))],
                        ins=[candL[j * NC * 128:(j + 1) * NC * 128, :].opt()],
                        outs=[candX[j * NC * 128:(j + 1) * NC * 128, :].opt()])
                    if "3" in phases:
                        phase3(j)


            if "3" not in phases:
                dummy = pool.tile([128, 2 * CV], F32, tag="dummy")
                nc.vector.memset(dummy[:], 0.0)
                for tt in range(NQT3):
                    nc.sync.dma_start(out=out[tt * 128:(tt + 1) * 128, :], in_=dummy[:])
            if "1" not in phases:
                # phase3-only: candL must still exist for collective; fill zero
                z = pool.tile([128, 2 * NCAND], F32, tag="z")
                nc.vector.memset(z[:], 0.0)
                for t in range(NQT):
                    nc.sync.dma_start(out=candL[t * 128:(t + 1) * 128, :], in_=z[:])
    nc.finalize()
    _prog_cache[phases] = nc
    return nc


def kernel(qk, mem_k, mem_v1, mem_v2, top_k):
    assert int(top_k) == TOPK
    qk = np.asarray(qk, dtype=np.float32)
    mem_k = np.asarray(mem_k, dtype=np.float32)
    mem_v1 = np.asarray(mem_v1, dtype=np.float32)
    mem_v2 = np.asarray(mem_v2, dtype=np.float32)

    q2 = qk.reshape(CK, Q)
    qTa = np.concatenate([q2 * 0.25, np.ones((1, Q), np.float32)], axis=0)
    a = np.sum(mem_k[0] * mem_k[0], axis=0, dtype=np.float32)  # [NE]
    vT = np.concatenate([mem_v1[0].T, mem_v2[0].T], axis=1).copy()  # [NE, 512]
    gbase = np.repeat(np.array(GB[:NGRP], dtype=np.float32), 8)
    prow256 = (np.arange(128, dtype=np.float32) * NSLOT).reshape(128, 1)

    in_maps = []
    for c in range(NC):
        sl = slice(c * NE_LOC, (c + 1) * NE_LOC)
        mkA = np.concatenate(
            [mem_k[0][:, sl], (-0.125 * a[sl])[None, :]], axis=0)
        in_maps.append({
            "qTa": qTa, "mkA": np.ascontiguousarray(mkA), "vT": vT,
            "gnc": np.broadcast_to(
                gbase + c * NE_LOC, (128, NCAND)).astype(np.float32).copy(),
            "prow256": prow256,
        })

    nc = _build_program()
    res = None
    for attempt in range(3):
        try:
            res = run_bass_kernel_spmd(nc, in_maps, core_ids=list(range(NC)))
            break
        except Exception:
            # transient device-unrecoverable states clear on the next attempt
            if attempt == 2:
                raise
            time.sleep(2.0)
    full = np.concatenate([res.results[c]["out"] for c in range(NC)], axis=0)
    return np.ascontiguousarray(full.T).reshape(1, 2 * CV, H, W)



# revision 6
# speedup vs baseline: 1.0101x; 1.0101x over previous
import sys, time
sys.path.insert(0, "/opt/trn_rl_repo")
import numpy as np
from concourse import bass, bacc, mybir, tile
from concourse.bass_utils import run_bass_kernel_spmd

# Problem constants (nn_Memory_88656714925588)
B, CK, CV = 1, 64, 256
H, W, T = 64, 64, 8
NE = H * W * T            # 32768
Q = H * W                 # 4096
NC = 8                    # cores
NE_LOC = NE // NC         # 4096 memory elements per core
Q_LOC = Q // NC           # 512 queries per core in phase 3
TOPK = 20
NGRP = 3                  # groups per query-tile in phase 1
GB = [0, 1366, 2732, 4096]  # uneven group bounds over NE_LOC
NCAND = NGRP * 8          # 24 candidates per (query, core)
NSLOT = NC * NCAND        # 256 candidates per query after all-gather
NQT = Q // 128            # 32 query tiles in phase 1
NQT3 = Q_LOC // 128       # 4 query tiles per core in phase 3
F32 = mybir.dt.float32
F32R = mybir.dt.float32r
U32 = mybir.dt.uint32
NEG = -1e30

_prog_cache = {}


def _build_program(phases="123"):
    if phases in _prog_cache:
        return _prog_cache[phases]
    nc = bacc.Bacc()
    qTa = nc.dram_tensor("qTa", [CK + 1, Q], F32R, kind="ExternalInput")
    mkA = nc.dram_tensor("mkA", [CK + 1, NE_LOC], F32R, kind="ExternalInput")
    vT = nc.dram_tensor("vT", [NE, 2 * CV], F32, kind="ExternalInput")
    gnc = nc.dram_tensor("gnc", [128, NCAND], F32, kind="ExternalInput")
    prow256 = nc.dram_tensor("prow256", [128, 1], F32, kind="ExternalInput")
    out = nc.dram_tensor("out", [Q_LOC, 2 * CV], F32, kind="ExternalOutput")

    with tile.TileContext(nc) as tc:
        with tc.tile_pool(name="sbuf", bufs=2) as pool, \
             tc.tile_pool(name="deep", bufs=10) as deep, \
             tc.tile_pool(name="affp", bufs=4) as affp, \
             tc.tile_pool(name="cst", bufs=1) as cst, \
             tc.tile_pool(name="psum", bufs=2, space="PSUM") as psum, \
             tc.tile_pool(name="dram", bufs=2, space="DRAM") as dram:

            qt = cst.tile([CK + 1, Q], F32R)
            mkt = cst.tile([CK + 1, NE_LOC], F32R)
            # chunked loads: first matmul needs only mkt[:, :512] and
            # qt[:, :128], so let compute start before the full MB lands
            for ci in range(8):
                nc.sync.dma_start(
                    out=mkt[:, ci * 512:(ci + 1) * 512],
                    in_=mkA[:, ci * 512:(ci + 1) * 512])
            for ci in range(4):
                nc.sync.dma_start(
                    out=qt[:, ci * 1024:(ci + 1) * 1024],
                    in_=qTa[:, ci * 1024:(ci + 1) * 1024])
            gb = cst.tile([128, NCAND], F32)
            nc.sync.dma_start(out=gb[:], in_=gnc[:])
            pr256 = cst.tile([128, 1], F32)
            nc.sync.dma_start(out=pr256[:], in_=prow256[:])

            candL = dram.tile([Q, 2 * NCAND], F32)
            candX = dram.tile([Q, 2 * NCAND], F32)

            # ---------------- Phase 3: merge + readout (q-sharded) --------
            def phase3(tt):
                cG = pool.tile([128, NC * 2 * NCAND], F32, tag="cG")
                nc.sync.dma_start(
                    out=cG[:],
                    in_=candX[tt * NC * 128:(tt + 1) * NC * 128, :]
                    .rearrange("(g p) c -> p g c", p=128))
                candQ = dram.tile([128 * NSLOT, 2], F32, tag="candQ")
                nc.sync.dma_start(
                    out=candQ[:].rearrange("(p u) two -> p (u two)", p=128),
                    in_=cG[:])
                # exact merge: 3 rounds of top-8 on the strided value view
                sv = cG[:].rearrange("p (u two) -> p u two", two=2)[:, :, 0]
                gvals = pool.tile([128, 24], F32, tag="gvals")
                gpos = pool.tile([128, 24], U32, tag="gpos")
                for r in range(3):
                    m8 = gvals[:, r * 8:(r + 1) * 8]
                    nc.vector.max(out=m8, in_=sv)
                    nc.vector.max_index(
                        out=gpos[:, r * 8:(r + 1) * 8], in_max=m8, in_values=sv)
                    if r < 2:
                        nc.vector.match_replace(
                            out=sv, in_to_replace=m8, in_values=sv, imm_value=NEG)
                # softmax over the top-20 values
                negm = pool.tile([128, 1], F32, tag="negm")
                nc.vector.tensor_scalar(
                    negm[:], gvals[:, 0:1], -1.0, None, op0=mybir.AluOpType.mult)
                wexp = pool.tile([128, TOPK], F32, tag="wexp")
                ssum = pool.tile([128, 1], F32, tag="ssum")
                nc.scalar.activation(
                    out=wexp[:], in_=gvals[:, :TOPK],
                    func=mybir.ActivationFunctionType.Exp,
                    bias=negm[:], scale=1.0, accum_out=ssum[:])
                rs = pool.tile([128, 1], F32, tag="rs")
                nc.vector.reciprocal(rs[:], ssum[:])
                wgt = pool.tile([128, TOPK], F32, tag="wgt")
                nc.vector.tensor_scalar(
                    wgt[:], wexp[:], rs[:], None, op0=mybir.AluOpType.mult)
                # winner pair offsets: row p of candQ-pairs = p*256 + pos
                posf = pool.tile([128, 24], F32, tag="posf")
                nc.vector.tensor_copy(posf[:], gpos[:])
                nc.vector.tensor_scalar(
                    posf[:], posf[:], pr256[:], None, op0=mybir.AluOpType.add)
                pou = pool.tile([128, 24], U32, tag="pou")
                nc.vector.tensor_copy(pou[:], posf[:])
                acc = pool.tile([128, 2 * CV], F32, tag="acc")
                nc.vector.memset(acc[:], 0.0)
                for k in range(TOPK):
                    pk = deep.tile([128, 2], F32, tag="pk")
                    nc.gpsimd.indirect_dma_start(
                        out=pk[:], out_offset=None, in_=candQ[:],
                        in_offset=bass.IndirectOffsetOnAxis(
                            ap=pou[:, k:k + 1], axis=0))
                    iku = deep.tile([128, 1], U32, tag="iku")
                    nc.scalar.copy(out=iku[:], in_=pk[:, 1:2])
                    gk = deep.tile([128, 2 * CV], F32, tag="gk")
                    nc.gpsimd.indirect_dma_start(
                        out=gk[:], out_offset=None, in_=vT[:],
                        in_offset=bass.IndirectOffsetOnAxis(ap=iku[:], axis=0))
                    nc.vector.scalar_tensor_tensor(
                        out=acc[:], in0=gk[:], scalar=wgt[:, k:k + 1], in1=acc[:],
                        op0=mybir.AluOpType.mult, op1=mybir.AluOpType.add)
                nc.sync.dma_start(
                    out=out[tt * 128:(tt + 1) * 128, :], in_=acc[:])


            # ---------------- Phase 1: local affinity + per-group top-8 ----
            # tile order: chunk-major (j, d) with t = d*NQT3 + j so each
            # chunk's AllToAll can fire as soon as its 8 tiles are done
            _order = [d * NQT3 + j for j in range(NQT3) for d in range(NC)]
            for ti, t in enumerate(_order[:NQT if "1" in phases else 0]):
                affs = affp.tile([128, NE_LOC], F32, tag="affs")
                cvals = pool.tile([128, NCAND], F32, tag="cvals", bufs=4)
                cidx = pool.tile([128, NCAND], U32, tag="cidx", bufs=4)
                for h in range(2):
                    ph = psum.tile([128, NE_LOC // 2], F32, tag="ph")
                    for c in range(4):
                        nc.tensor.matmul(
                            out=ph[:, c * 512:(c + 1) * 512],
                            lhsT=qt[:, t * 128:(t + 1) * 128],
                            rhs=mkt[:, h * 2048 + c * 512: h * 2048 + (c + 1) * 512],
                            start=True, stop=True)
                    nc.scalar.copy(out=affs[:, h * 2048:(h + 1) * 2048], in_=ph[:])
                    for g in range(NGRP):
                        if not (GB[g] < (h + 1) * 2048 and GB[g + 1] > h * 2048
                                and GB[g + 1] <= (h + 1) * 2048):
                            continue
                        sl = affs[:, GB[g]:GB[g + 1]]
                        nc.vector.max(out=cvals[:, g * 8:(g + 1) * 8], in_=sl)
                        nc.vector.max_index(
                            out=cidx[:, g * 8:(g + 1) * 8],
                            in_max=cvals[:, g * 8:(g + 1) * 8], in_values=sl)
                crow = pool.tile([128, 2 * NCAND], F32, tag="crow", bufs=4)
                cr3 = crow[:].rearrange("p (u two) -> p u two", two=2)
                nc.scalar.copy(out=cr3[:, :, 0], in_=cvals[:])
                nc.vector.scalar_tensor_tensor(
                    out=cr3[:, :, 1], in0=cidx[:], scalar=1.0, in1=gb[:],
                    op0=mybir.AluOpType.mult, op1=mybir.AluOpType.add)
                j, d = t % NQT3, t // NQT3
                row = (j * NC + d) * 128
                nc.sync.dma_start(
                    out=candL[row:row + 128, :], in_=crow[:])
                if "2" in phases and ti % NC == NC - 1:
                    nc.gpsimd.collective_compute(
                        "AllToAll", mybir.AluOpType.bypass,
                        replica_groups=[list(range(NC))],
                        ins=[candL[j * NC * 128:(j + 1) * NC * 128, :].opt()],
                        outs=[candX[j * NC * 128:(j + 1) * NC * 128, :].opt()])
                    if "3" in phases:
                        phase3(j)


            if "3" not in phases:
                dummy = pool.tile([128, 2 * CV], F32, tag="dummy")
                nc.vector.memset(dummy[:], 0.0)
                for tt in range(NQT3):
                    nc.sync.dma_start(out=out[tt * 128:(tt + 1) * 128, :], in_=dummy[:])
            if "1" not in phases:
                # phase3-only: candL must still exist for collective; fill zero
                z = pool.tile([128, 2 * NCAND], F32, tag="z")
                nc.vector.memset(z[:], 0.0)
                for t in range(NQT):
                    nc.sync.dma_start(out=candL[t * 128:(t + 1) * 128, :], in_=z[:])
    nc.finalize()
    _prog_cache[phases] = nc
    return nc


def _host_inputs(qk, mem_k, mem_v1, mem_v2, top_k=TOPK):
    qk = np.asarray(qk, dtype=np.float32)
    mem_k = np.asarray(mem_k, dtype=np.float32)
    mem_v1 = np.asarray(mem_v1, dtype=np.float32)
    mem_v2 = np.asarray(mem_v2, dtype=np.float32)

    q2 = qk.reshape(CK, Q)
    qTa = np.concatenate([q2 * 0.25, np.ones((1, Q), np.float32)], axis=0)
    a = np.sum(mem_k[0] * mem_k[0], axis=0, dtype=np.float32)  # [NE]
    vT = np.concatenate([mem_v1[0].T, mem_v2[0].T], axis=1).copy()  # [NE, 512]
    gbase = np.repeat(np.array(GB[:NGRP], dtype=np.float32), 8)
    prow256 = (np.arange(128, dtype=np.float32) * NSLOT).reshape(128, 1)

    in_maps = []
    for c in range(NC):
        sl = slice(c * NE_LOC, (c + 1) * NE_LOC)
        mkA = np.concatenate(
            [mem_k[0][:, sl], (-0.125 * a[sl])[None, :]], axis=0)
        in_maps.append({
            "qTa": qTa, "mkA": np.ascontiguousarray(mkA), "vT": vT,
            "gnc": np.broadcast_to(
                gbase + c * NE_LOC, (128, NCAND)).astype(np.float32).copy(),
            "prow256": prow256,
        })
    return in_maps


def _assemble_output(outs):
    full = np.concatenate(outs, axis=0)
    return np.ascontiguousarray(full.T).reshape(1, 2 * CV, H, W)


def kernel(qk, mem_k, mem_v1, mem_v2, top_k):
    assert int(top_k) == TOPK
    in_maps = _host_inputs(qk, mem_k, mem_v1, mem_v2)
    nc = _build_program()
    res = None
    for attempt in range(3):
        try:
            res = run_bass_kernel_spmd(nc, in_maps, core_ids=list(range(NC)))
            break
        except Exception:
            # transient device-unrecoverable states clear on the next attempt
            if attempt == 2:
                raise
            time.sleep(2.0)
    return _assemble_output([res.results[c]["out"] for c in range(NC)])



# revision 7
# speedup vs baseline: 1.6894x; 1.6725x over previous
import sys, time
sys.path.insert(0, "/opt/trn_rl_repo")
import numpy as np
from concourse import bass, bacc, mybir, tile
from concourse.bass_utils import run_bass_kernel_spmd

# Problem constants (nn_Memory_88656714925588)
B, CK, CV = 1, 64, 256
H, W, T = 64, 64, 8
NE = H * W * T            # 32768 memory elements
Q = H * W * 64 // 64      # 4096 queries
NC = 8                    # cores
Q_LOC = Q // NC           # 512 queries per core (query-sharded)
NQT = Q_LOC // 128        # 4 query tiles per core
TOPK = 20
CW = 64                   # chunk width for the screen
NCH = NE // CW            # 512 chunks per query row
NSEL = 24                 # chunks selected per query (>= 20 guarantees coverage)
NCAND = NSEL * 8          # 192 candidates after per-chunk top-8
NSLICE = 8                # 4096-column slices per tile
SLW = NE // NSLICE        # 4096
F32 = mybir.dt.float32
F16 = mybir.dt.float16
U32 = mybir.dt.uint32
NEG = -1e30
EPS = 2.0 ** -17

_prog_cache = {}


def _build_program():
    if "p" in _prog_cache:
        return _prog_cache["p"]
    nc = bacc.Bacc()
    qTb = nc.dram_tensor("qTb", [CK + 2, Q_LOC], F16, kind="ExternalInput")
    mkB = nc.dram_tensor("mkB", [CK + 2, NE], F16, kind="ExternalInput")
    vTb = nc.dram_tensor("vTb", [NE, 2 * CV], F16, kind="ExternalInput")
    prow512 = nc.dram_tensor("prow512", [128, 1], F32, kind="ExternalInput")
    prow192 = nc.dram_tensor("prow192", [128, 1], F32, kind="ExternalInput")
    eps512 = nc.dram_tensor("eps512", [128, NCH], F32, kind="ExternalInput")
    eps192 = nc.dram_tensor("eps192", [128, NCAND], F32, kind="ExternalInput")
    out = nc.dram_tensor("out", [Q_LOC, 2 * CV], F32, kind="ExternalOutput")

    with tile.TileContext(nc) as tc:
        with tc.tile_pool(name="cst", bufs=1) as cst, \
             tc.tile_pool(name="aff", bufs=3) as affp, \
             tc.tile_pool(name="tree", bufs=2) as tre, \
             tc.tile_pool(name="sel", bufs=2) as sel, \
             tc.tile_pool(name="gat", bufs=2) as gat, \
             tc.tile_pool(name="psum", bufs=2, space="PSUM") as psum, \
             tc.tile_pool(name="dram", bufs=2, space="DRAM") as dram:

            qt = cst.tile([CK + 2, Q_LOC], F16)
            mkt = cst.tile([CK + 2, NE], F16)
            # chunked mk load so the first matmuls start early
            for ci in range(NSLICE):
                nc.sync.dma_start(
                    out=mkt[:, ci * SLW:(ci + 1) * SLW],
                    in_=mkB[:, ci * SLW:(ci + 1) * SLW])
            nc.sync.dma_start(out=qt[:], in_=qTb[:])
            pr512 = cst.tile([128, 1], F32)
            nc.sync.dma_start(out=pr512[:], in_=prow512[:])
            pr192 = cst.tile([128, 1], F32)
            nc.sync.dma_start(out=pr192[:], in_=prow192[:])
            ep512 = cst.tile([128, NCH], F32)
            nc.sync.dma_start(out=ep512[:], in_=eps512[:])
            ep192 = cst.tile([128, NCAND], F32)
            nc.sync.dma_start(out=ep192[:], in_=eps192[:])

            for t in range(NQT):
                affsD = dram.tile([128 * NCH, CW], F16, tag="affsD")
                affsDv = affsD[:].rearrange("(p c) w -> p (c w)", p=128)
                elD = dram.tile([128 * NCAND, 1], F32, tag="elD")
                cmax = tre.tile([128, NCH], F16, tag="cmax")

                for s in range(NSLICE):
                    aff4 = affp.tile([128, SLW], F16, tag="aff4")
                    for h in range(2):
                        ph = psum.tile([128, 2048], F32, tag="ph")
                        for c in range(4):
                            col = s * SLW + h * 2048 + c * 512
                            nc.tensor.matmul(
                                out=ph[:, c * 512:(c + 1) * 512],
                                lhsT=qt[:, t * 128:(t + 1) * 128],
                                rhs=mkt[:, col:col + 512],
                                start=True, stop=True)
                        nc.scalar.activation(
                            out=aff4[:, h * 2048:(h + 1) * 2048], in_=ph[:],
                            func=mybir.ActivationFunctionType.Copy)
                    # stage this slice to DRAM for the per-query rescan gathers
                    eng = nc.sync if s % 2 == 0 else nc.gpsimd
                    eng.dma_start(
                        out=affsDv[:, s * SLW:(s + 1) * SLW], in_=aff4[:])
                    # chunk-local pairwise-max tree: 4096 -> 64 chunk maxima
                    a3 = aff4[:].rearrange("p (g w) -> p g w", w=CW)
                    t1 = tre.tile([128, 2048], F16, tag="t1")
                    nc.vector.tensor_tensor(
                        out=t1[:].rearrange("p (g w) -> p g w", w=32),
                        in0=a3[:, :, 0:32], in1=a3[:, :, 32:64],
                        op=mybir.AluOpType.max)
                    t2 = tre.tile([128, 1024], F16, tag="t2")
                    nc.vector.tensor_tensor(
                        out=t2[:].rearrange("p (g w) -> p g w", w=16),
                        in0=t1[:].rearrange("p (g w) -> p g w", w=32)[:, :, 0:16],
                        in1=t1[:].rearrange("p (g w) -> p g w", w=32)[:, :, 16:32],
                        op=mybir.AluOpType.max)
                    t3 = tre.tile([128, 512], F16, tag="t3")
                    nc.vector.tensor_tensor(
                        out=t3[:].rearrange("p (g w) -> p g w", w=8),
                        in0=t2[:].rearrange("p (g w) -> p g w", w=16)[:, :, 0:8],
                        in1=t2[:].rearrange("p (g w) -> p g w", w=16)[:, :, 8:16],
                        op=mybir.AluOpType.max)
                    t4 = tre.tile([128, 256], F16, tag="t4")
                    nc.vector.tensor_tensor(
                        out=t4[:].rearrange("p (g w) -> p g w", w=4),
                        in0=t3[:].rearrange("p (g w) -> p g w", w=8)[:, :, 0:4],
                        in1=t3[:].rearrange("p (g w) -> p g w", w=8)[:, :, 4:8],
                        op=mybir.AluOpType.max)
                    t5 = tre.tile([128, 128], F16, tag="t5")
                    nc.vector.tensor_tensor(
                        out=t5[:].rearrange("p (g w) -> p g w", w=2),
                        in0=t4[:].rearrange("p (g w) -> p g w", w=4)[:, :, 0:2],
                        in1=t4[:].rearrange("p (g w) -> p g w", w=4)[:, :, 2:4],
                        op=mybir.AluOpType.max)
                    nc.vector.tensor_tensor(
                        out=cmax[:, s * 64:(s + 1) * 64],
                        in0=t5[:].rearrange("p (g w) -> p g w", w=2)[:, :, 0],
                        in1=t5[:].rearrange("p (g w) -> p g w", w=2)[:, :, 1],
                        op=mybir.AluOpType.max)

                # ---- select top-NSEL chunks per query (tie-free in f32) ----
                cmaxf = sel.tile([128, NCH], F32, tag="cmaxf")
                nc.vector.tensor_copy(cmaxf[:], cmax[:])
                nc.vector.tensor_tensor(
                    out=cmaxf[:], in0=cmaxf[:], in1=ep512[:],
                    op=mybir.AluOpType.add)
                cidu = sel.tile([128, NSEL], U32, tag="cidu")
                m8 = sel.tile([128, 8], F32, tag="m8")
                for r in range(NSEL // 8):
                    nc.vector.max(out=m8[:], in_=cmaxf[:])
                    nc.vector.max_index(
                        out=cidu[:, r * 8:(r + 1) * 8], in_max=m8[:],
                        in_values=cmaxf[:])
                    if r < NSEL // 8 - 1:
                        nc.vector.match_replace(
                            out=cmaxf[:], in_to_replace=m8[:],
                            in_values=cmaxf[:], imm_value=NEG)
                cidf = sel.tile([128, NSEL], F32, tag="cidf")
                nc.vector.tensor_copy(cidf[:], cidu[:])
                offf = sel.tile([128, NSEL], F32, tag="offf")
                nc.vector.tensor_scalar(
                    offf[:], cidf[:], pr512[:], None, op0=mybir.AluOpType.add)
                offu = sel.tile([128, NSEL], U32, tag="offu")
                nc.vector.tensor_copy(offu[:], offf[:])

                # ---- gather the selected chunks, rescan for top-8 each ----
                g24 = gat.tile([128, NSEL * CW], F16, tag="g24")
                nc.gpsimd.indirect_dma_start(
                    out=g24[:].rearrange("p (k w) -> p k w", w=CW),
                    out_offset=None, in_=affsD[:],
                    in_offset=bass.IndirectOffsetOnAxis(ap=offu[:], axis=0))
                cv8 = sel.tile([128, NCAND], F16, tag="cv8")
                pix = sel.tile([128, NCAND], U32, tag="pix")
                for j in range(NSEL):
                    nc.vector.max(
                        out=cv8[:, j * 8:(j + 1) * 8],
                        in_=g24[:, j * CW:(j + 1) * CW])
                    nc.vector.max_index(
                        out=pix[:, j * 8:(j + 1) * 8],
                        in_max=cv8[:, j * 8:(j + 1) * 8],
                        in_values=g24[:, j * CW:(j + 1) * CW])
                # decode element index: el = cid*64 + pix
                pixf = sel.tile([128, NCAND], F32, tag="pixf")
                nc.vector.tensor_copy(pixf[:], pix[:])
                elf = sel.tile([128, NCAND], F32, tag="elf")
                nc.vector.scalar_tensor_tensor(
                    out=elf[:].rearrange("p (k r) -> p k r", r=8),
                    in0=cidf[:].rearrange("p (k u) -> p k u", u=1)
                    .broadcast_to([128, NSEL, 8]),
                    scalar=float(CW),
                    in1=pixf[:].rearrange("p (k r) -> p k r", r=8),
                    op0=mybir.AluOpType.mult, op1=mybir.AluOpType.add)
                nc.sync.dma_start(
                    out=elD[:].rearrange("(p u) one -> p (u one)", p=128),
                    in_=elf[:])

                # ---- merge: exact top-20 of the 192 candidates ----
                cvf = sel.tile([128, NCAND], F32, tag="cvf")
                nc.vector.tensor_copy(cvf[:], cv8[:])
                nc.vector.tensor_tensor(
                    out=cvf[:], in0=cvf[:], in1=ep192[:],
                    op=mybir.AluOpType.add)
                gvals = sel.tile([128, 24], F32, tag="gvals")
                gpos = sel.tile([128, 24], U32, tag="gpos")
                for r in range(3):
                    g8 = gvals[:, r * 8:(r + 1) * 8]
                    nc.vector.max(out=g8, in_=cvf[:])
                    nc.vector.max_index(
                        out=gpos[:, r * 8:(r + 1) * 8], in_max=g8,
                        in_values=cvf[:])
                    if r < 2:
                        nc.vector.match_replace(
                            out=cvf[:], in_to_replace=g8, in_values=cvf[:],
                            imm_value=NEG)
                gposf = sel.tile([128, TOPK], F32, tag="gposf")
                nc.vector.tensor_copy(gposf[:], gpos[:, :TOPK])
                off2 = sel.tile([128, TOPK], F32, tag="off2")
                nc.vector.tensor_scalar(
                    off2[:], gposf[:], pr192[:], None, op0=mybir.AluOpType.add)
                offu2 = sel.tile([128, TOPK], U32, tag="offu2")
                nc.vector.tensor_copy(offu2[:], off2[:])
                pk = sel.tile([128, TOPK], F32, tag="pk")
                nc.gpsimd.indirect_dma_start(
                    out=pk[:], out_offset=None, in_=elD[:],
                    in_offset=bass.IndirectOffsetOnAxis(ap=offu2[:], axis=0))
                iku = sel.tile([128, TOPK], U32, tag="iku")
                nc.vector.tensor_copy(iku[:], pk[:])

                # ---- softmax over the top-20 values ----
                negm = sel.tile([128, 1], F32, tag="negm")
                nc.vector.tensor_scalar(
                    negm[:], gvals[:, 0:1], -1.0, None,
                    op0=mybir.AluOpType.mult)
                wexp = sel.tile([128, TOPK], F32, tag="wexp")
                ssum = sel.tile([128, 1], F32, tag="ssum")
                nc.scalar.activation(
                    out=wexp[:], in_=gvals[:, :TOPK],
                    func=mybir.ActivationFunctionType.Exp,
                    bias=negm[:], scale=1.0, accum_out=ssum[:])
                rs = sel.tile([128, 1], F32, tag="rs")
                nc.vector.reciprocal(rs[:], ssum[:])
                wgt = sel.tile([128, TOPK], F32, tag="wgt")
                nc.vector.tensor_scalar(
                    wgt[:], wexp[:], rs[:], None, op0=mybir.AluOpType.mult)

                # ---- gather V rows (one batched indirect DMA), readout ----
                vTg = gat.tile([128, TOPK * 2 * CV], F16, tag="vTg")
                nc.gpsimd.indirect_dma_start(
                    out=vTg[:].rearrange("p (k c) -> p k c", c=2 * CV),
                    out_offset=None, in_=vTb[:],
                    in_offset=bass.IndirectOffsetOnAxis(ap=iku[:], axis=0))
                acc = gat.tile([128, 2 * CV], F32, tag="acc")
                nc.vector.memset(acc[:], 0.0)
                for k in range(TOPK):
                    nc.vector.scalar_tensor_tensor(
                        out=acc[:], in0=vTg[:, k * 2 * CV:(k + 1) * 2 * CV],
                        scalar=wgt[:, k:k + 1], in1=acc[:],
                        op0=mybir.AluOpType.mult, op1=mybir.AluOpType.add)
                nc.sync.dma_start(
                    out=out[t * 128:(t + 1) * 128, :], in_=acc[:])
    nc.finalize()
    _prog_cache["p"] = nc
    return nc


def _host_inputs(qk, mem_k, mem_v1, mem_v2, top_k=TOPK):
    qk = np.asarray(qk, dtype=np.float32)
    mem_k = np.asarray(mem_k, dtype=np.float32)
    mem_v1 = np.asarray(mem_v1, dtype=np.float32)
    mem_v2 = np.asarray(mem_v2, dtype=np.float32)

    q2 = qk.reshape(CK, Q)
    a = np.sum(mem_k[0] * mem_k[0], axis=0, dtype=np.float32)      # [NE]
    na = -0.125 * a
    nh = na.astype(np.float16).astype(np.float32)
    nl = (na - nh).astype(np.float16)
    mkB = np.concatenate(
        [mem_k[0].astype(np.float16), nh.astype(np.float16)[None, :],
         nl[None, :]], axis=0)                                      # [66, NE]
    vTb = np.concatenate(
        [mem_v1[0].T, mem_v2[0].T], axis=1).astype(np.float16)      # [NE, 512]
    prow512 = (np.arange(128, dtype=np.float32) * NCH).reshape(128, 1)
    prow192 = (np.arange(128, dtype=np.float32) * NCAND).reshape(128, 1)
    eps512 = np.broadcast_to(
        np.arange(NCH, dtype=np.float32) * EPS, (128, NCH)).copy()
    eps192 = np.broadcast_to(
        np.arange(NCAND, dtype=np.float32) * EPS, (128, NCAND)).copy()

    in_maps = []
    for c in range(NC):
        sl = slice(c * Q_LOC, (c + 1) * Q_LOC)
        qTb = np.concatenate(
            [(0.25 * q2[:, sl]).astype(np.float16),
             np.ones((2, Q_LOC), np.float16)], axis=0)              # [66, 512]
        in_maps.append({
            "qTb": qTb, "mkB": mkB, "vTb": vTb,
            "prow512": prow512, "prow192": prow192,
            "eps512": eps512, "eps192": eps192,
        })
    return in_maps


def _assemble_output(outs):
    full = np.concatenate(outs, axis=0)
    return np.ascontiguousarray(full.T).reshape(1, 2 * CV, H, W)


def kernel(qk, mem_k, mem_v1, mem_v2, top_k):
    assert int(top_k) == TOPK
    in_maps = _host_inputs(qk, mem_k, mem_v1, mem_v2)
    nc = _build_program()
    res = None
    for attempt in range(3):
        try:
            res = run_bass_kernel_spmd(nc, in_maps, core_ids=list(range(NC)))
            break
        except Exception:
            # transient device-unrecoverable states clear on the next attempt
            if attempt == 2:
                raise
            time.sleep(2.0)
    return _assemble_output([res.results[c]["out"] for c in range(NC)])
